# revision 19
# baseline (speedup 1.0000x reference)
# Trainium2 Bass kernel for nn_Attention_80779744903426
#
# Reference computation (b=4, n=2048, c=1024, h=16, d=64):
#   qkv = x @ w_qkv ; split to q,k,v per head
#   attn = softmax(q k^T / sqrt(c)) ; out = (attn v) concat ; y = out @ w_proj + b_proj
#
# Sharding (8 cores): data-parallel over batch (4) x tensor-parallel over
# head-groups (2 groups of 8 heads, Megatron-style). Each core computes a
# partial y for its batch from its 8 heads; host sums the two partials per
# batch and adds b_proj.
#
# Per-core program (all matmuls bf16, fp32 PSUM accumulation):
#   A) qk^T = wqk^T @ x^T staged to DRAM bf16 (Q^T rows 0:512, K^T rows
#      512:1024); V = x @ wv -> SBUF bf16 with a ones column appended.
#      Pass 1 = V + K^T/Q^T of head-pair 0; the remaining projection chains
#      are interleaved into pair-0's attention iterations so the PE fills
#      the ACT(exp)-bound stretches.
#   B) per head pair and q-chunk of 512, software-pipelined:
#      S^T[k,q] = K^T_h(stationary, row-tiled K=64, A/B heads interleaved on
#      row groups) x Q^T_h(moving); exp via ACT over 3-bank PSUM batches
#      (softmax scale folded into the activation), bf16 out;
#      O'[65,q] = [V_h | 1]^T @ P~^T over 16 k-tiles (ones column = fused
#      softmax denominator). PV of the previous (pair,chunk) is interleaved
#      between S batches of the current one so the in-order PE queue never
#      stalls (stalls re-throttle the PE clock via HAM). Normalization:
#      copy O' out of PSUM, fast-reciprocal of the sums row (partition 0),
#      partition-broadcast via a DRAM-bounce DMA on the gpsimd queue,
#      multiply.
#   C) y = O^T(stationary) @ wp(moving) over 4 o-tiles; interleaved into the
#      last pair's iterations per q-chunk.

import numpy as np

DIM = 1024
N = 2048
B = 4
NH = 16
HD = 64
SCALE = 1.0 / DIM**0.5

HPC = 8            # heads per core
PAIRS = HPC // 2   # head pairs (row-tiled together)
CT = 8             # contraction tiles over c=1024
NT = 16            # n tiles of 128
ACH = 512          # phase-A n-chunk
QCH = 512          # phase-B q-chunk
NQC = N // QCH     # 4 q-chunks
KT = 16            # k tiles of 128 in attention

S_BATCHES = [(0, 3), (3, 3), (6, 3), (9, 3), (12, 2), (14, 2)]

_CACHE = {}


def _build_nc():
    import concourse.bass as bass
    from concourse import bacc, mybir, tile

    f32 = mybir.dt.float32
    bf16 = mybir.dt.bfloat16
    EXP = mybir.ActivationFunctionType.Exp

    nc = bacc.Bacc("TRN2", target_bir_lowering=False, debug=False)

    xT_d = nc.dram_tensor("xT", [DIM, N], bf16, kind="ExternalInput").ap()
    wqk_d = nc.dram_tensor("wqk", [DIM, 1024], bf16, kind="ExternalInput").ap()
    wv_d = nc.dram_tensor("wv", [DIM, 512], bf16, kind="ExternalInput").ap()
    wp_d = nc.dram_tensor("wp", [512, DIM], bf16, kind="ExternalInput").ap()
    y_d = nc.dram_tensor("y", [N, DIM], f32, kind="ExternalOutput").ap()

    with tile.TileContext(nc) as tc:
        with (
            tc.tile_pool(name="p16", bufs=5) as p16,      # 16KB slots: ptiles / xt
            tc.tile_pool(name="wqk", bufs=1) as wqkp,
            tc.tile_pool(name="wv", bufs=1) as wvp,
            tc.tile_pool(name="wp", bufs=1) as wpp,
            tc.tile_pool(name="v", bufs=1) as vp,
            tc.tile_pool(name="ot", bufs=1) as otp,
            tc.tile_pool(name="kt", bufs=2) as ktp,
            tc.tile_pool(name="qt", bufs=2) as qtp,
            tc.tile_pool(name="misc", bufs=2) as miscp,
            tc.tile_pool(name="ps", bufs=1, space="PSUM") as psp,
            tc.tile_pool(name="dram", bufs=1, space="DRAM") as dp,
        ):
            qkT_d = dp.tile([DIM, N], bf16, name="qkT_stage")
            # ---- static tiles ----
            wqk_sb = wqkp.tile([128, CT, 1024], bf16)
            for ct in range(CT):
                nc.sync.dma_start(wqk_sb[:, ct, :], wqk_d[128 * ct : 128 * (ct + 1), :])
            wv_sb = wvp.tile([128, CT, 512], bf16)
            for ct in range(CT):
                nc.sync.dma_start(wv_sb[:, ct, :], wv_d[128 * ct : 128 * (ct + 1), :])
            wp_sb = wpp.tile([128, 4, 1024], bf16)
            for ot in range(4):
                nc.sync.dma_start(wp_sb[:, ot, :], wp_d[128 * ot : 128 * (ot + 1), :])

            v_sb = vp.tile([128, NT, HPC, HD + 1], bf16)  # [k-part, k-tile, head, d | 1]
            nc.vector.memset(v_sb[:, :, :, HD], 1.0)

            ot_sb = otp.tile([128, PAIRS, N], bf16)  # O^T rows: pair p = rows 128p..

            xT_r = xT_d.rearrange("(t p) n -> p t n", p=128)

            # ---- phase A helpers ----
            def emit_qkt_chain(xt, mt, ach):
                qps = psp.tile([128, 512], f32, tag="acc", bufs=2, name="qps")
                for ct in range(CT):
                    nc.tensor.matmul(qps, wqk_sb[:, ct, 128 * mt : 128 * (mt + 1)],
                                     xt[:, ct, :], start=(ct == 0), stop=(ct == CT - 1))
                stg = miscp.tile([128, 512], bf16, tag="stg", bufs=3, name="stg")
                nc.vector.tensor_copy(stg, qps)
                nc.sync.dma_start(
                    qkT_d[128 * mt : 128 * (mt + 1), ACH * ach : ACH * (ach + 1)], stg
                )

            def load_xt(ach):
                xt = p16.tile([128, CT, ACH], bf16, tag="big16", name="xt")
                nc.sync.dma_start(xt, xT_r[:, :, ACH * ach : ACH * (ach + 1)])
                return xt

            def emit_v_group_on(xt, ach):
                for sub in range(ACH // 128):
                    nt = (ACH // 128) * ach + sub
                    vps = psp.tile([128, 512], f32, tag="acc", bufs=2, name="vps")
                    for ct in range(CT):
                        nc.tensor.matmul(vps, xt[:, ct, 128 * sub : 128 * (sub + 1)],
                                         wv_sb[:, ct, :], start=(ct == 0),
                                         stop=(ct == CT - 1))
                    nc.vector.tensor_copy(
                        v_sb[:, nt, :, 0:HD],
                        vps.rearrange("p (h d) -> p h d", h=HPC),
                    )

            # ---- phase A pass 1: K^T pair0 + V + Q^T(pair0, chunk0) ----
            # Just enough to start attention; everything else is spread as
            # filler into the attention iterations below.
            p1_xts = []
            for ach in range(N // ACH):
                xt = load_xt(ach)
                p1_xts.append(xt)
                emit_qkt_chain(xt, 4, ach)
            for ach in range(N // ACH):
                emit_v_group_on(p1_xts[ach], ach)
            emit_qkt_chain(p1_xts[0], 0, 0)

            # ---- phase B: qc-outer so proj chunks unlock early ----
            PV_SEGS = [(0, 3), (3, 3), (6, 3), (9, 3), (12, 2), (14, 2)]

            def emit_pv_segment(st, seg):
                p0, ptl, opsl = st
                k0, kn = PV_SEGS[seg]
                for hh in range(2):
                    h = 2 * p0 + hh
                    for k in range(k0, k0 + kn):
                        nc.tensor.matmul(opsl[hh], v_sb[:, k, h, :],
                                         ptl[hh][:, k, :],
                                         start=(k == 0), stop=(k == KT - 1))

            def emit_norm(st, qc0):
                # Copy O' out of PSUM first so the PSUM slot recycles without
                # waiting for the reciprocal/broadcast chain. Bounce DMAs ride
                # the gpsimd SWDGE queue so they never head-of-line-block the
                # sync queue carrying bulk loads.
                p0, ptl, opsl = st
                for hh in range(2):
                    ops = opsl[hh]
                    ostg = miscp.tile([HD, QCH], f32, tag="ostg", bufs=4,
                                      name="ostg")
                    nc.vector.tensor_copy(ostg, ops[0:HD, :])
                    # denominator row staged to partition 0: the custom-DVE
                    # reciprocal_approx_fast misreads non-zero base partitions
                    den = miscp.tile([1, QCH], f32, tag="den", bufs=4, name="den")
                    nc.vector.tensor_copy(den, ops[HD : HD + 1, :])
                    rcp = miscp.tile([1, QCH], f32, tag="rcp", bufs=4, name="rcp")
                    nc.vector.reciprocal_approx_fast(rcp, den)
                    rcp_d = dp.tile([1, QCH], f32, tag="rcpd", bufs=4, name="rcpd")
                    nc.gpsimd.dma_start(rcp_d, rcp)
                    bc = miscp.tile([64, QCH], f32, tag="bc", bufs=4, name="bc")
                    rap = rcp_d[:]
                    nc.gpsimd.dma_start(
                        bc,
                        bass.AP(tensor=rap.tensor, offset=rap.offset,
                                ap=[[0, 64]] + list(rap.ap[1:])),
                    )
                    nc.vector.tensor_mul(
                        ot_sb[64 * hh : 64 * (hh + 1), p0, QCH * qc0 : QCH * (qc0 + 1)],
                        ostg,
                        bc,
                    )

            def emit_proj_half(qc0, half):
                # y columns for q-chunk qc0 (needs ot_sb[:, :, chunk] complete)
                sub = half
                nt = (QCH // 128) * qc0 + sub * 2
                for nt2 in (nt, nt + 1):
                    for yc in range(2):
                        yps = psp.tile([128, 512], f32, tag="acc", bufs=2, name="yps")
                        for ot in range(4):
                            nc.tensor.matmul(
                                yps, ot_sb[:, ot, 128 * nt2 : 128 * (nt2 + 1)],
                                wp_sb[:, ot, 512 * yc : 512 * (yc + 1)],
                                start=(ot == 0), stop=(ot == 3))
                        stg = miscp.tile([128, 512], f32, tag="ystg", bufs=2,
                                         name="ystg")
                        nc.vector.tensor_copy(stg, yps)
                        nc.sync.dma_start(
                            y_d[128 * nt2 : 128 * (nt2 + 1), 512 * yc : 512 * (yc + 1)],
                            stg,
                        )

            def qkt_thunk(ach, mts):
                def t():
                    xt = load_xt(ach)
                    for mt in mts:
                        emit_qkt_chain(xt, mt, ach)
                return t

            def proj_thunk(qc0, half):
                return lambda: emit_proj_half(qc0, half)

            # filler thunks per iteration index (iter = 4*qc + p):
            # K^T pair p+1 must be fully staged before iter p+1 (kt DMA);
            # Q^T (mt p, chunk qc) before iter 4qc+p; proj(qc) after the
            # norm of (qc, pair3), which is emitted in iter 4qc+4.
            EXTRAS = {
                0: [qkt_thunk(0, [5, 1]), qkt_thunk(1, [5]), qkt_thunk(2, [5]),
                    qkt_thunk(3, [5])],
                1: [qkt_thunk(0, [6, 2]), qkt_thunk(1, [6]), qkt_thunk(2, [6]),
                    qkt_thunk(3, [6])],
                2: [qkt_thunk(0, [7, 3]), qkt_thunk(1, [7]), qkt_thunk(2, [7]),
                    qkt_thunk(3, [7])],
                3: [qkt_thunk(1, [0, 1]), qkt_thunk(1, [2, 3])],
                4: [qkt_thunk(2, [0, 1]), qkt_thunk(2, [2, 3])],
                5: [qkt_thunk(3, [0, 1]), qkt_thunk(3, [2, 3]),
                    proj_thunk(0, 0), proj_thunk(0, 1)],
                9: [proj_thunk(1, 0), proj_thunk(1, 1)],
                13: [proj_thunk(2, 0), proj_thunk(2, 1)],
            }

            kt_tiles = {}
            pv_st = None
            pv_qc = None
            it = -1
            for qc in range(NQC):
                for p in range(PAIRS):
                    it += 1
                    if p not in kt_tiles:
                        kt_sb = ktp.tile([128, N], bf16, tag="kt", bufs=4, name=f"kt{p}")
                        nc.sync.dma_start(
                            kt_sb, qkT_d[512 + 128 * p : 512 + 128 * (p + 1), :]
                        )
                        kt_tiles[p] = kt_sb
                    kt_sb = kt_tiles[p]
                    qt_sb = qtp.tile([128, QCH], bf16, name="qt_sb")
                    nc.sync.dma_start(
                        qt_sb, qkT_d[128 * p : 128 * (p + 1), QCH * qc : QCH * (qc + 1)]
                    )
                    extras = EXTRAS.get(it, [])
                    ptiles = [
                        p16.tile([128, KT, QCH], bf16, tag="big16", name=f"pt{hh}")
                        for hh in range(2)
                    ]
                    for bi, (b0, bn) in enumerate(S_BATCHES):
                        sps = [
                            psp.tile([128, 3, QCH], f32, tag="sb3", bufs=2,
                                     name=f"sps{hh}")
                            for hh in range(2)
                        ]
                        for i in range(bn):
                            k = b0 + i
                            for hh in range(2):
                                sl = slice(64 * hh, 64 * (hh + 1))
                                nc.tensor.matmul(
                                    sps[hh][:, i, :],
                                    kt_sb[sl, 128 * k : 128 * (k + 1)],
                                    qt_sb[sl, :], start=True, stop=True)
                        for hh in range(2):
                            nc.scalar.activation(
                                out=ptiles[hh][:, b0 : b0 + bn, :],
                                in_=sps[hh][:, 0:bn, :],
                                func=EXP,
                                scale=float(SCALE),
                            )
                        if pv_st is not None:
                            emit_pv_segment(pv_st, bi)
                        if bi < len(extras):
                            extras[bi]()
                    if pv_st is not None:
                        emit_norm(pv_st, pv_qc)
                    opsl = [
                        psp.tile([HD + 1, QCH], f32, tag="acc", bufs=2,
                                 name=f"ops{hh}")
                        for hh in range(2)
                    ]
                    pv_st = (p, ptiles, opsl)
                    pv_qc = qc
            # drain the last (qc3, pair3)
            for seg in range(len(PV_SEGS)):
                emit_pv_segment(pv_st, seg)
            emit_norm(pv_st, pv_qc)
            emit_proj_half(3, 0)
            emit_proj_half(3, 1)

    nc.compile()
    return nc


def get_nc():
    if "nc" not in _CACHE:
        _CACHE["nc"] = _build_nc()
    return _CACHE["nc"]


def make_in_maps(x, w_qkv, w_proj):
    import ml_dtypes

    bf = ml_dtypes.bfloat16
    in_maps = []
    for c in range(8):
        b, g = c // 2, c % 2
        in_maps.append({
            "xT": np.ascontiguousarray(x[b].T).astype(bf),
            "wqk": np.ascontiguousarray(
                np.concatenate(
                    [w_qkv[:, 512 * g : 512 * (g + 1)],
                     w_qkv[:, 1024 + 512 * g : 1024 + 512 * (g + 1)]], axis=1
                )).astype(bf),
            "wv": np.ascontiguousarray(
                w_qkv[:, 2048 + 512 * g : 2048 + 512 * (g + 1)]).astype(bf),
            "wp": np.ascontiguousarray(
                w_proj[512 * g : 512 * (g + 1), :]).astype(bf),
        })
    return in_maps


def kernel(x, w_qkv, w_proj, b_proj):
    from concourse.bass_utils import run_bass_kernel_spmd

    x = np.asarray(x, dtype=np.float32)
    w_qkv = np.asarray(w_qkv, dtype=np.float32)
    w_proj = np.asarray(w_proj, dtype=np.float32)
    b_proj = np.asarray(b_proj, dtype=np.float32)

    nc = get_nc()
    in_maps = make_in_maps(x, w_qkv, w_proj)
    res = run_bass_kernel_spmd(nc, in_maps, list(range(8))).results

    out = np.zeros((B, N, DIM), dtype=np.float32)
    for c in range(8):
        out[c // 2] += res[c]["y"]
    return out + b_proj
